# revision 26
# baseline (speedup 1.0000x reference)
"""Causal self-attention (B=4, T=2048, C=1024, H=16) on 8 Trainium2 NeuronCores.

Sharding: 8 cores = 4 batches x 2 head-groups. Core c handles batch c//2 and
heads 8*(c%2) .. 8*(c%2)+8 (512 of the 1024 channels). Each core computes the
QKV projection for its channels over its batch's 2048 tokens, flash-style
causal attention for its 8 heads, and a partial output projection over its
512 c_proj input rows. The host sums the two partials per batch and adds the
bias terms (b_proj plus the b_v contribution, which is w_proj @ b_v because
softmax rows sum to one; b_k shifts every logit in a row equally so softmax
drops it; b_q and the 1/sqrt(hd) scale are folded into the q weights host-side).

Precision: everything runs bf16 into fp32 PSUM accumulation (1 PE cycle/row;
fp32r costs ~4x at full clock). Host-side study: bf16 inputs end-to-end give
3.7e-3 max-rel error vs the 2e-2 gate. Output is stored bf16, upcast on host.

Schedule: the attention inner loop is paced by ScalarE (exp ~500ns/block vs
PE's ~430ns S+PV) — measured by work-doubling probes (2x exp costs its full
duration; 2x QKV matmul adds ~1:1). So all non-attention PE work (QKV
projection tiles, output-projection tiles) is threaded INTO the attention
block loop as single-matmul "fillers" that execute in PE's per-block idle
window while ScalarE streams exps. Head-pair g's attention pumps the QKV
tiles of pair g+1; the output projection for token-chunk c is pumped during
the last pair once all heads' chunk-c normalizations are done. V tiles and
pair-0 qk tiles are emitted as small bursts between pair-0's chunks (their
consumers are too close for filler pacing).

Device layout (per core):
  x    [128, 8, 2048] bf16 resident; wqk/wv/wp resident (loaded once/rep)
  qkT  [128, 8, 2048] bf16: blocks 0-3 = scaled q channels, 4-7 = k
  V    [128, 16, 8, 65] bf16 token-major with a ones column so the PV matmul
       row 64 accumulates the softmax denominator
  S^T  [k,q] psum via matmul(lhsT=kT, rhs=qT) (K=64); head pairs sit at
       partition bases 0/64. Exp on ScalarE writes P^T bf16; causal masking
       uses persistent band tiles with structurally-zero prefixes plus a
       triangular multiply on diagonal subtiles (Pool engine).
  Normalization is immediate and SBUF-local: DVE reciprocal of the [1,512]
  denominator row, K=1 ones-matmul broadcast, tensor_tensor multiply.

This container's walrus accepts only one hardware wait slot per instruction,
so after Tile scheduling we split multi-wait sync_info into standalone
EventSemaphore waits (_split_multiwaits).
"""

import sys

if '/opt/trn_rl_repo' not in sys.path:
    sys.path.insert(0, '/opt/trn_rl_repo')

import numpy as np

B, T, C, H = 4, 2048, 1024, 16
HD = C // H            # 64
HPC = 8                # heads per core
CL = HPC * HD          # 512 local channels
NCORES = 8

_cache = {}


def _split_multiwaits(nc, max_waits=1):
    import concourse.mybir as mybir
    n = 0
    ctr = [0]
    for fn in nc.m.functions:
        for bb in fn.blocks:
            out = []
            for inst in bb.instructions:
                si = inst.sync_info
                if si is not None and si.on_wait and len(si.on_wait) > max_waits:
                    waits = list(si.on_wait)
                    head, tail = waits[:-max_waits], waits[-max_waits:]
                    for w in head:
                        ctr[0] += 1
                        out.append(mybir.InstEventSemaphore(
                            name=f"wsplit-{ctr[0]}",
                            engine=inst.engine,
                            ins=[], outs=[],
                            sync_info=mybir.SyncInfo(on_wait=[w], on_update=[]),
                        ))
                    inst.sync_info = mybir.SyncInfo(
                        on_wait=tail, on_update=list(si.on_update))
                    n += 1
                out.append(inst)
            bb.instructions[:] = out
    return n


def _build(reps=1, phases=None):
    import contextlib
    import concourse.bass as bass
    import concourse.mybir as mybir
    import concourse.tile as tile

    f32 = mybir.dt.float32
    bf16 = mybir.dt.bfloat16
    Act = mybir.ActivationFunctionType
    Alu = mybir.AluOpType

    nc = bass.Bass()

    xT_d = nc.dram_tensor("xT", [C, T], bf16, kind="ExternalInput")
    wqk_d = nc.dram_tensor("wqk", [C, 2 * CL], bf16, kind="ExternalInput")
    wv_d = nc.dram_tensor("wv", [C, CL], bf16, kind="ExternalInput")
    bq_d = nc.dram_tensor("bq", [128, 4], f32, kind="ExternalInput")
    wp_d = nc.dram_tensor("wp", [CL, C], bf16, kind="ExternalInput")
    mask_d = nc.dram_tensor("mask", [128, 128], bf16, kind="ExternalInput")
    out_d = nc.dram_tensor("out", [T, C], bf16, kind="ExternalOutput")

    NQ = T // 512      # 4 q-chunks of 512 tokens
    NT = T // 128      # 16 token-blocks

    with tile.TileContext(nc) as tc:
        with tc.tile_pool(name="persist", bufs=1) as persist, \
             tc.tile_pool(name="pt", bufs=4) as ptpool, \
             tc.tile_pool(name="ysb", bufs=4) as ypool, \
             tc.tile_pool(name="outst", bufs=2) as opool, \
             tc.tile_pool(name="rst", bufs=2) as rpool, \
             tc.tile_pool(name="psmm", bufs=2, space="PSUM") as psmm, \
             tc.tile_pool(name="psst", bufs=2, space="PSUM") as psst, \
             tc.tile_pool(name="psy", bufs=2, space="PSUM") as psy:

            with (tc.For_i(0, reps, 1) if reps > 1 else contextlib.nullcontext()):
                xall = persist.tile([128, 8, T], bf16, tag="xall")
                wqk_sb = persist.tile([128, 8, 2 * CL], bf16, tag="wqk")
                wv_sb = persist.tile([128, 8, CL], bf16, tag="wv")
                wp_sb = persist.tile([128, 4, C], bf16, tag="wp")
                qkT = persist.tile([128, 8, T], bf16, tag="qkT")
                yN = persist.tile([128, 4, T], bf16, tag="yN")
                V = persist.tile([128, NT, HPC, HD + 1], bf16, tag="V")
                bq_sb = persist.tile([128, 4], f32, tag="bq")
                mask_sb = persist.tile([128, 128], bf16, tag="mask")
                ones64 = persist.tile([1, 64], bf16, tag="ones")
                # 512-wide bands: right-only diagonal exps (q-sub-chunk local)
                bands = [[persist.tile([128, 512], bf16, tag=f"band{s}{j}",
                                       name=f"band{s}{j}")
                          for j in (1, 2, 3)] for s in (0, 1)]
                # 1024-wide bands: merged (two-q-chunk) diagonal exps
                bands2 = [[persist.tile([128, 1024], bf16, tag=f"bnd2{s}{j}",
                                        name=f"bnd2{s}{j}")
                           for j in (1, 2, 3)] for s in (0, 1)]

                nc.sync.dma_start(bq_sb[:], bq_d[:])
                nc.sync.dma_start(mask_sb[:], mask_d[:])
                nc.sync.dma_start(
                    xall[:], xT_d[:].rearrange("(j p) n -> p j n", p=128))
                nc.sync.dma_start(
                    wqk_sb[:], wqk_d[:].rearrange("(j p) m -> p j m", p=128))
                nc.sync.dma_start(
                    wv_sb[:], wv_d[:].rearrange("(j p) m -> p j m", p=128))
                nc.sync.dma_start(
                    wp_sb[:], wp_d[:].rearrange("(mq p) oc -> p mq oc", p=128))
                nc.vector.memset(ones64[:], 1.0)
                nc.vector.memset(V[:, :, :, HD:HD + 1], 1.0)
                for s in (0, 1):
                    for jr in (1, 2, 3):
                        nc.vector.memset(bands[s][jr - 1][:, 0:128 * jr], 0.0)
                        nc.vector.memset(bands2[s][jr - 1][:, 0:128 * jr], 0.0)

                # ---------- tile emitters (burst form) ----------
                def emit_qk_tile(m, tq):
                    ts512 = tq * 512
                    ps = psmm.tile([128, 512], f32, tag="mm")
                    for j in range(8):
                        nc.tensor.matmul(
                            ps[:], lhsT=wqk_sb[:, j, m * 128:m * 128 + 128],
                            rhs=xall[:, j, ts512:ts512 + 512],
                            start=(j == 0), stop=(j == 7))
                    if m < 4:  # q block: add bias (scale pre-folded)
                        nc.vector.tensor_scalar_add(
                            qkT[:, m, ts512:ts512 + 512], ps[:],
                            bq_sb[:, m:m + 1])
                    else:      # k block: plain evacuate
                        nc.vector.tensor_copy(
                            qkT[:, m, ts512:ts512 + 512], ps[:])

                def emit_v_tile(tb):
                    ps = psmm.tile([128, 512], f32, tag="mm")
                    for j in range(8):
                        nc.tensor.matmul(
                            ps[:], lhsT=xall[:, j, tb * 128:tb * 128 + 128],
                            rhs=wv_sb[:, j, :],
                            start=(j == 0), stop=(j == 7))
                    nc.vector.tensor_copy(
                        V[:, tb, :, 0:HD],
                        ps[:].rearrange("p (h d) -> p h d", h=HPC))

                # ---------- filler queue: single-matmul closures that the
                # ---------- attention loop pumps into PE's idle windows
                fillers = []

                def queue_qk_tile(m, tq):
                    ts512 = tq * 512
                    box = [None]

                    def step(j):
                        def f():
                            if j == 0:
                                box[0] = psmm.tile([128, 512], f32, tag="mm",
                                                   name=f"qkf{m}{tq}")
                            nc.tensor.matmul(
                                box[0][:],
                                lhsT=wqk_sb[:, j, m * 128:m * 128 + 128],
                                rhs=xall[:, j, ts512:ts512 + 512],
                                start=(j == 0), stop=(j == 7))
                            if j == 7:
                                if m < 4:
                                    nc.vector.tensor_scalar_add(
                                        qkT[:, m, ts512:ts512 + 512],
                                        box[0][:], bq_sb[:, m:m + 1])
                                else:
                                    nc.vector.tensor_copy(
                                        qkT[:, m, ts512:ts512 + 512], box[0][:])
                        return f
                    fillers.extend(step(j) for j in range(8))

                ost_box = {}

                def queue_proj_tile(o, t):
                    os_ = o * 512
                    box = [None]

                    def step(mq):
                        def f():
                            if mq == 0:
                                box[0] = psmm.tile([128, 512], f32, tag="mm",
                                                   name=f"pjf{o}{t}")
                            nc.tensor.matmul(
                                box[0][:],
                                lhsT=yN[:, mq, t * 128:t * 128 + 128],
                                rhs=wp_sb[:, mq, os_:os_ + 512],
                                start=(mq == 0), stop=(mq == 3))
                            if mq == 3:
                                # both 512-col halves share one [128,1024]
                                # staging tile; DMA once per token-block
                                if t not in ost_box:
                                    ost_box[t] = opool.tile(
                                        [128, 1024], bf16, tag="outst",
                                        name=f"ost{t}")
                                ost = ost_box[t]
                                nc.vector.tensor_copy(
                                    ost[:, os_:os_ + 512], box[0][:])
                                if o == 1:
                                    nc.sync.dma_start(
                                        out_d[t * 128:t * 128 + 128, :],
                                        ost[:])
                                    del ost_box[t]
                        return f
                    fillers.extend(step(mq) for mq in range(4))

                def pump(n=1):
                    for _ in range(min(n, len(fillers))):
                        fillers.pop(0)()

                # ---------- immediate per-chunk normalization: reciprocal of
                # ---------- the denominator row, K=1 ones-matmul broadcast
                # ---------- (ps_rep shares the psy "y" ring), multiply
                def emit_norm(h, c, y_c):
                    m = h // 2
                    pb = 64 * (h % 2)
                    cs = c * 512
                    rrow = rpool.tile([1, 512], bf16, tag="rrow",
                                      name=f"rr{h}{c}")
                    with nc.allow_low_precision(
                            reason="1/denom in bf16; ~0.4% y scale error is "
                                   "well inside the 2e-2 gate"):
                        nc.vector.reciprocal(rrow[:], y_c[64:65, :])
                    ps_rep = psy.tile([64, 512], f32, tag="y",
                                      name=f"rep{h}{c}")
                    nc.tensor.matmul(ps_rep[:], lhsT=ones64[:],
                                     rhs=rrow[:], start=True, stop=True)
                    nc.vector.tensor_tensor(
                        yN[pb:pb + 64, m, cs:cs + 512],
                        y_c[0:64, :], ps_rep[:], Alu.mult)

                # ---------- attention chunk-PAIR (q cols 1024p..1024p+1024):
                # ---------- each k-block's exp covers both 512-col q-chunks
                # ---------- in ONE ScalarE instruction (S matmuls fill the
                # ---------- two bank-halves of a [128,1024] psum tile); one
                # ---------- filler pumped per PV
                def emit_attn_pair(h, p):
                    m = h // 2
                    s = h % 2
                    pb = 64 * s
                    cl, cr = 2 * p, 2 * p + 1          # chunk indices
                    qs = 1024 * p
                    jmax_left = 8 * p + 3
                    nj = 8 * p + 8
                    ys_l = psy.tile([65, 512], f32, tag="y",
                                    name=f"ysl{h}{p}")
                    ys_r = psy.tile([65, 512], f32, tag="y",
                                    name=f"ysr{h}{p}")
                    y_cl = ypool.tile([65, 512], f32, tag="ysb",
                                      name=f"ycl{h}{p}")
                    y_cr = ypool.tile([65, 512], f32, tag="ysb",
                                      name=f"ycr{h}{p}")
                    pv_q = []          # deferred PV ops: (j, pt, side)
                    for j in range(nj):
                        left = j <= jmax_left
                        ps_st = psst.tile([128, 1024], f32, tag="st",
                                          name=f"st{h}{p}{j}")
                        if left:
                            # two matmuls fill the two bank-halves (a single
                            # 1024-wide psum write faults the PE: matmul
                            # output cannot cross a PSUM bank)
                            nc.tensor.matmul(
                                ps_st[:, 0:512],
                                lhsT=qkT[pb:pb + 64, 4 + m,
                                         j * 128:j * 128 + 128],
                                rhs=qkT[pb:pb + 64, m, qs:qs + 512],
                                start=True, stop=True)
                            nc.tensor.matmul(
                                ps_st[:, 512:1024],
                                lhsT=qkT[pb:pb + 64, 4 + m,
                                         j * 128:j * 128 + 128],
                                rhs=qkT[pb:pb + 64, m, qs + 512:qs + 1024],
                                start=True, stop=True)
                            jrel = j - 8 * p
                            if jrel <= 0:
                                pt = ptpool.tile([128, 1024], bf16, tag="pt",
                                                 name=f"pt{h}{p}{j}")
                                nc.scalar.activation(pt[:], ps_st[:], Act.Exp)
                            else:
                                pt = bands2[s][jrel - 1]
                                z = 128 * jrel
                                nc.scalar.activation(
                                    pt[:, z:1024], ps_st[:, z:1024], Act.Exp)
                            if jrel >= 0:
                                z = 128 * jrel
                                nc.gpsimd.tensor_tensor(
                                    pt[:, z:z + 128], pt[:, z:z + 128],
                                    mask_sb[:], Alu.mult)
                        else:
                            nc.tensor.matmul(
                                ps_st[:, 512:1024],
                                lhsT=qkT[pb:pb + 64, 4 + m,
                                         j * 128:j * 128 + 128],
                                rhs=qkT[pb:pb + 64, m, qs + 512:qs + 1024],
                                start=True, stop=True)
                            jrel = j - (8 * p + 4)     # right-chunk diagonal
                            if jrel <= 0:
                                pt = ptpool.tile([128, 1024], bf16, tag="pt",
                                                 name=f"pt{h}{p}{j}")
                                nc.scalar.activation(
                                    pt[:, 512:1024], ps_st[:, 512:1024],
                                    Act.Exp)
                            else:
                                pt = bands[s][jrel - 1]
                                z = 128 * jrel
                                nc.scalar.activation(
                                    pt[:, z:512], ps_st[:, 512 + z:1024],
                                    Act.Exp)
                            if jrel >= 0:
                                z = 128 * jrel
                                if pt.shape[1] == 1024:
                                    nc.gpsimd.tensor_tensor(
                                        pt[:, 512 + z:512 + z + 128],
                                        pt[:, 512 + z:512 + z + 128],
                                        mask_sb[:], Alu.mult)
                                else:
                                    nc.gpsimd.tensor_tensor(
                                        pt[:, z:z + 128], pt[:, z:z + 128],
                                        mask_sb[:], Alu.mult)

                        # drain the deferred-PV queue one block behind the
                        # exp frontier so PE never waits on a fresh exp
                        while pv_q and pv_q[0][0] < j:
                            pj, ppt, side = pv_q.pop(0)
                            pump(1)
                            if side == 'l':
                                nc.tensor.matmul(
                                    ys_l[:], lhsT=V[:, pj, h, :],
                                    rhs=ppt[:, 0:512],
                                    start=(pj == 0), stop=(pj == jmax_left))
                            else:
                                rw = (ppt[:, 512:1024]
                                      if ppt.shape[1] == 1024 else ppt[:])
                                nc.tensor.matmul(
                                    ys_r[:], lhsT=V[:, pj, h, :], rhs=rw,
                                    start=(pj == 0), stop=(pj == nj - 1))
                                if pj == jmax_left:
                                    # left accumulator complete: evacuate and
                                    # normalize mid-pair
                                    nc.vector.tensor_copy(y_cl[:], ys_l[:])
                                    emit_norm(h, cl, y_cl)
                        if left:
                            pv_q.append((j, pt, 'l'))
                        pv_q.append((j, pt, 'r'))
                    for pj, ppt, side in pv_q:
                        pump(1)
                        if side == 'l':
                            nc.tensor.matmul(
                                ys_l[:], lhsT=V[:, pj, h, :],
                                rhs=ppt[:, 0:512],
                                start=(pj == 0), stop=(pj == jmax_left))
                        else:
                            rw = (ppt[:, 512:1024]
                                  if ppt.shape[1] == 1024 else ppt[:])
                            nc.tensor.matmul(
                                ys_r[:], lhsT=V[:, pj, h, :], rhs=rw,
                                start=(pj == 0), stop=(pj == nj - 1))
                            if pj == jmax_left:
                                nc.vector.tensor_copy(y_cl[:], ys_l[:])
                                emit_norm(h, cl, y_cl)
                    nc.vector.tensor_copy(y_cr[:], ys_r[:])
                    emit_norm(h, cr, y_cr)

                # ---------- schedule ----------
                # preamble: qk quarters 0-1 + V token-blocks 0-7 (everything
                # the first head-pair's chunk-pair 0 touches)
                emit_qk_tile(0, 0)
                emit_qk_tile(4, 0)
                emit_qk_tile(0, 1)
                emit_qk_tile(4, 1)
                for tb in range(8):
                    emit_v_tile(tb)

                for g in range(4):          # head pairs (2g, 2g+1)
                    if g > 0:
                        # fillers for this pair were queued during pair g-1;
                        # drain any leftovers before their consumers
                        while fillers:
                            pump(1)
                    if g < 3:
                        for mm in (g + 1, 4 + g + 1):
                            for tq in range(4):
                                queue_qk_tile(mm, tq)
                    emit_attn_pair(2 * g, 0)
                    emit_attn_pair(2 * g + 1, 0)
                    if g == 0:
                        # pair-0's own later quarters + V blocks (burst:
                        # consumers are too close for filler pacing)
                        emit_qk_tile(0, 2)
                        emit_qk_tile(4, 2)
                        emit_qk_tile(0, 3)
                        emit_qk_tile(4, 3)
                        for tb in range(8, 16):
                            emit_v_tile(tb)
                    if g == 3:
                        # chunks 0-1 of all heads normalized after h7 pair 0
                        for t in range(8):
                            queue_proj_tile(0, t)
                            queue_proj_tile(1, t)
                    emit_attn_pair(2 * g, 1)
                    emit_attn_pair(2 * g + 1, 1)

                # chunks 2-3 of all heads are now normalized
                for t in range(8, 16):
                    queue_proj_tile(0, t)
                    queue_proj_tile(1, t)
                # drain remaining projection fillers
                while fillers:
                    pump(1)

    nsplit = _split_multiwaits(nc)
    return nc, nsplit


def _prep_inputs(x, w_attn, b_attn, w_proj):
    """Per-core input maps. Core c: batch c//2, head-group c%2."""
    import ml_dtypes
    bf = ml_dtypes.bfloat16
    x = np.ascontiguousarray(x, dtype=np.float32)
    w_attn = np.asarray(w_attn, dtype=np.float32)
    b_attn = np.asarray(b_attn, dtype=np.float32)
    w_proj = np.asarray(w_proj, dtype=np.float32)
    scale = np.float32(1.0 / np.sqrt(HD))

    mask = (np.arange(128)[:, None] <= np.arange(128)[None, :]).astype(bf)

    in_maps = []
    for core in range(NCORES):
        b = core // 2
        g = core % 2
        gc = CL * g
        wq = w_attn[gc:gc + CL, :] * scale          # [512, 1024]
        wk = w_attn[C + gc:C + gc + CL, :]
        wv = w_attn[2 * C + gc:2 * C + gc + CL, :]
        bq = b_attn[gc:gc + CL] * scale
        in_maps.append({
            "xT": np.ascontiguousarray(x[b].T).astype(bf),
            "wqk": np.ascontiguousarray(
                np.concatenate([wq.T, wk.T], axis=1)).astype(bf),
            "wv": np.ascontiguousarray(wv.T).astype(bf),
            "bq": np.ascontiguousarray(bq.reshape(4, 128).T),
            "wp": np.ascontiguousarray(
                w_proj[:, gc:gc + CL].T.astype(bf)),
            "mask": mask,
        })
    return in_maps


def _run(in_maps, reps=1):
    from concourse.bass_utils import run_bass_kernel_spmd
    key = reps
    if key not in _cache:
        _cache[key] = _build(reps)
    nc, _ = _cache[key]
    return run_bass_kernel_spmd(nc, in_maps, list(range(NCORES)))


def kernel(x, w_attn, b_attn, w_proj, b_proj):
    x = np.asarray(x, dtype=np.float32)
    w_attn = np.asarray(w_attn, dtype=np.float32)
    b_attn = np.asarray(b_attn, dtype=np.float32)
    w_proj = np.asarray(w_proj, dtype=np.float32)
    b_proj = np.asarray(b_proj, dtype=np.float32)

    in_maps = _prep_inputs(x, w_attn, b_attn, w_proj)
    res = _run(in_maps).results

    # host-side unshard: sum the two head-group partials per batch and add
    # the bias terms (b_proj + w_proj @ b_v; softmax rows sum to 1).
    bv = b_attn[2 * C:]
    const = (w_proj @ bv + b_proj).astype(np.float32)
    out = np.empty((B, T, C), dtype=np.float32)
    for b in range(B):
        out[b] = (res[2 * b]["out"].astype(np.float32)
                  + res[2 * b + 1]["out"].astype(np.float32) + const)
    return out


# revision 27
# speedup vs baseline: 1.0702x; 1.0702x over previous
"""Causal self-attention (B=4, T=2048, C=1024, H=16) on 8 Trainium2 NeuronCores.

Sharding: 8 cores = 4 batches x 2 head-groups. Core c handles batch c//2 and
heads 8*(c%2) .. 8*(c%2)+8 (512 of the 1024 channels). Each core computes the
QKV projection for its channels over its batch's 2048 tokens, flash-style
causal attention for its 8 heads, and a partial output projection over its
512 c_proj input rows. The host sums the two partials per batch and adds the
bias terms (b_proj plus the b_v contribution, which is w_proj @ b_v because
softmax rows sum to one; b_k shifts every logit in a row equally so softmax
drops it; b_q and the 1/sqrt(hd) scale are folded into the q weights host-side).

Precision: everything runs bf16 into fp32 PSUM accumulation (1 PE cycle/row;
fp32r costs ~4x at full clock). Host-side study: bf16 inputs end-to-end give
3.7e-3 max-rel error vs the 2e-2 gate. Output is stored bf16, upcast on host.

Schedule: the attention inner loop is paced by ScalarE — work-doubling
probes measured ~850ns of FIXED cost per Activation instruction (decode +
split semaphore waits) on top of ~1 col/cycle compute, so exp instruction
COUNT is the dominant knob. Attention therefore processes q-chunk PAIRS:
each k-block's two [128,512] S matmuls fill the two bank-halves of one
[128,1024] psum tile and a SINGLE exp covers both chunks (192 exps instead
of 320, zero extra element work; a single 1024-wide matmul would be nicer
still but a matmul's psum write cannot cross a PSUM bank — it faults the
PE). All non-attention PE work (QKV projection tiles, output-projection
tiles) is threaded INTO the attention block loop as single-matmul "fillers"
that execute in PE's per-block idle window while ScalarE streams exps:
head-pair g's attention pumps the QKV tiles of pair g+1, and the output
projection is pumped during the last pair as that chunk's normalizations
complete. V tiles and pair-0 qk tiles are emitted as small bursts around
pair-0 (their consumers are too close for filler pacing).

Device layout (per core):
  x    [128, 8, 2048] bf16 resident; wqk/wv/wp resident (loaded once/rep)
  qkT  [128, 8, 2048] bf16: blocks 0-3 = scaled q channels, 4-7 = k
  V    [128, 16, 8, 65] bf16 token-major with a ones column so the PV matmul
       row 64 accumulates the softmax denominator
  S^T  [k,q] psum via matmul(lhsT=kT, rhs=qT) (K=64); head pairs sit at
       partition bases 0/64. Exp on ScalarE writes P^T bf16; causal masking
       uses persistent band tiles with structurally-zero prefixes plus a
       triangular multiply on diagonal subtiles (Pool engine).
  Normalization is immediate and SBUF-local: DVE reciprocal of the [1,512]
  denominator row, K=1 ones-matmul broadcast, tensor_tensor multiply.

This container's walrus accepts only one hardware wait slot per instruction,
so after Tile scheduling we split multi-wait sync_info into standalone
EventSemaphore waits (_split_multiwaits).
"""

import sys

if '/opt/trn_rl_repo' not in sys.path:
    sys.path.insert(0, '/opt/trn_rl_repo')

import numpy as np

B, T, C, H = 4, 2048, 1024, 16
HD = C // H            # 64
HPC = 8                # heads per core
CL = HPC * HD          # 512 local channels
NCORES = 8

_cache = {}


def _split_multiwaits(nc, max_waits=1):
    import concourse.mybir as mybir
    n = 0
    ctr = [0]
    for fn in nc.m.functions:
        for bb in fn.blocks:
            out = []
            for inst in bb.instructions:
                si = inst.sync_info
                if si is not None and si.on_wait and len(si.on_wait) > max_waits:
                    waits = list(si.on_wait)
                    head, tail = waits[:-max_waits], waits[-max_waits:]
                    for w in head:
                        ctr[0] += 1
                        out.append(mybir.InstEventSemaphore(
                            name=f"wsplit-{ctr[0]}",
                            engine=inst.engine,
                            ins=[], outs=[],
                            sync_info=mybir.SyncInfo(on_wait=[w], on_update=[]),
                        ))
                    inst.sync_info = mybir.SyncInfo(
                        on_wait=tail, on_update=list(si.on_update))
                    n += 1
                out.append(inst)
            bb.instructions[:] = out
    return n


def _build(reps=1, phases=None):
    import contextlib
    import concourse.bass as bass
    import concourse.mybir as mybir
    import concourse.tile as tile

    f32 = mybir.dt.float32
    bf16 = mybir.dt.bfloat16
    Act = mybir.ActivationFunctionType
    Alu = mybir.AluOpType

    nc = bass.Bass()

    xT_d = nc.dram_tensor("xT", [C, T], bf16, kind="ExternalInput")
    wqk_d = nc.dram_tensor("wqk", [C, 2 * CL], bf16, kind="ExternalInput")
    wv_d = nc.dram_tensor("wv", [C, CL], bf16, kind="ExternalInput")
    bq_d = nc.dram_tensor("bq", [128, 4], f32, kind="ExternalInput")
    wp_d = nc.dram_tensor("wp", [CL, C], bf16, kind="ExternalInput")
    mask_d = nc.dram_tensor("mask", [128, 128], bf16, kind="ExternalInput")
    out_d = nc.dram_tensor("out", [T, C], bf16, kind="ExternalOutput")

    NQ = T // 512      # 4 q-chunks of 512 tokens
    NT = T // 128      # 16 token-blocks

    with tile.TileContext(nc) as tc:
        with tc.tile_pool(name="persist", bufs=1) as persist, \
             tc.tile_pool(name="pt", bufs=4) as ptpool, \
             tc.tile_pool(name="ysb", bufs=4) as ypool, \
             tc.tile_pool(name="outst", bufs=2) as opool, \
             tc.tile_pool(name="rst", bufs=2) as rpool, \
             tc.tile_pool(name="psmm", bufs=2, space="PSUM") as psmm, \
             tc.tile_pool(name="psst", bufs=2, space="PSUM") as psst, \
             tc.tile_pool(name="psy", bufs=2, space="PSUM") as psy:

            with (tc.For_i(0, reps, 1) if reps > 1 else contextlib.nullcontext()):
                xall = persist.tile([128, 8, T], bf16, tag="xall")
                wqk_sb = persist.tile([128, 8, 2 * CL], bf16, tag="wqk")
                wv_sb = persist.tile([128, 8, CL], bf16, tag="wv")
                wp_sb = persist.tile([128, 4, C], bf16, tag="wp")
                qkT = persist.tile([128, 8, T], bf16, tag="qkT")
                yN = persist.tile([128, 4, T], bf16, tag="yN")
                V = persist.tile([128, NT, HPC, HD + 1], bf16, tag="V")
                bq_sb = persist.tile([128, 4], f32, tag="bq")
                mask_sb = persist.tile([128, 128], bf16, tag="mask")
                ones64 = persist.tile([1, 64], bf16, tag="ones")
                # 512-wide bands: right-only diagonal exps (q-sub-chunk local)
                bands = [[persist.tile([128, 512], bf16, tag=f"band{s}{j}",
                                       name=f"band{s}{j}")
                          for j in (1, 2, 3)] for s in (0, 1)]
                # 1024-wide bands: merged (two-q-chunk) diagonal exps
                bands2 = [[persist.tile([128, 1024], bf16, tag=f"bnd2{s}{j}",
                                        name=f"bnd2{s}{j}")
                           for j in (1, 2, 3)] for s in (0, 1)]

                nc.sync.dma_start(bq_sb[:], bq_d[:])
                nc.sync.dma_start(mask_sb[:], mask_d[:])
                nc.sync.dma_start(
                    xall[:], xT_d[:].rearrange("(j p) n -> p j n", p=128))
                nc.sync.dma_start(
                    wqk_sb[:], wqk_d[:].rearrange("(j p) m -> p j m", p=128))
                nc.sync.dma_start(
                    wv_sb[:], wv_d[:].rearrange("(j p) m -> p j m", p=128))
                nc.sync.dma_start(
                    wp_sb[:], wp_d[:].rearrange("(mq p) oc -> p mq oc", p=128))
                nc.vector.memset(ones64[:], 1.0)
                nc.vector.memset(V[:, :, :, HD:HD + 1], 1.0)
                for s in (0, 1):
                    for jr in (1, 2, 3):
                        nc.vector.memset(bands[s][jr - 1][:, 0:128 * jr], 0.0)
                        nc.vector.memset(bands2[s][jr - 1][:, 0:128 * jr], 0.0)

                # ---------- tile emitters (burst form) ----------
                def emit_qk_tile(m, tq):
                    ts512 = tq * 512
                    ps = psmm.tile([128, 512], f32, tag="mm")
                    for j in range(8):
                        nc.tensor.matmul(
                            ps[:], lhsT=wqk_sb[:, j, m * 128:m * 128 + 128],
                            rhs=xall[:, j, ts512:ts512 + 512],
                            start=(j == 0), stop=(j == 7))
                    if m < 4:  # q block: add bias (scale pre-folded)
                        nc.vector.tensor_scalar_add(
                            qkT[:, m, ts512:ts512 + 512], ps[:],
                            bq_sb[:, m:m + 1])
                    else:      # k block: plain evacuate
                        nc.vector.tensor_copy(
                            qkT[:, m, ts512:ts512 + 512], ps[:])

                def emit_v_tile(tb):
                    ps = psmm.tile([128, 512], f32, tag="mm")
                    for j in range(8):
                        nc.tensor.matmul(
                            ps[:], lhsT=xall[:, j, tb * 128:tb * 128 + 128],
                            rhs=wv_sb[:, j, :],
                            start=(j == 0), stop=(j == 7))
                    nc.vector.tensor_copy(
                        V[:, tb, :, 0:HD],
                        ps[:].rearrange("p (h d) -> p h d", h=HPC))

                # ---------- filler queue: single-matmul closures that the
                # ---------- attention loop pumps into PE's idle windows
                fillers = []

                def queue_qk_tile(m, tq):
                    ts512 = tq * 512
                    box = [None]

                    def step(j):
                        def f():
                            if j == 0:
                                box[0] = psmm.tile([128, 512], f32, tag="mm",
                                                   name=f"qkf{m}{tq}")
                            nc.tensor.matmul(
                                box[0][:],
                                lhsT=wqk_sb[:, j, m * 128:m * 128 + 128],
                                rhs=xall[:, j, ts512:ts512 + 512],
                                start=(j == 0), stop=(j == 7))
                            if j == 7:
                                if m < 4:
                                    nc.vector.tensor_scalar_add(
                                        qkT[:, m, ts512:ts512 + 512],
                                        box[0][:], bq_sb[:, m:m + 1])
                                else:
                                    nc.vector.tensor_copy(
                                        qkT[:, m, ts512:ts512 + 512], box[0][:])
                        return f
                    fillers.extend(step(j) for j in range(8))

                ost_box = {}

                def queue_proj_tile(o, t):
                    os_ = o * 512
                    box = [None]

                    def step(mq):
                        def f():
                            if mq == 0:
                                box[0] = psmm.tile([128, 512], f32, tag="mm",
                                                   name=f"pjf{o}{t}")
                            nc.tensor.matmul(
                                box[0][:],
                                lhsT=yN[:, mq, t * 128:t * 128 + 128],
                                rhs=wp_sb[:, mq, os_:os_ + 512],
                                start=(mq == 0), stop=(mq == 3))
                            if mq == 3:
                                # both 512-col halves share one [128,1024]
                                # staging tile; DMA once per token-block
                                if t not in ost_box:
                                    ost_box[t] = opool.tile(
                                        [128, 1024], bf16, tag="outst",
                                        name=f"ost{t}")
                                ost = ost_box[t]
                                nc.vector.tensor_copy(
                                    ost[:, os_:os_ + 512], box[0][:])
                                if o == 1:
                                    nc.sync.dma_start(
                                        out_d[t * 128:t * 128 + 128, :],
                                        ost[:])
                                    del ost_box[t]
                        return f
                    fillers.extend(step(mq) for mq in range(4))

                def pump(n=1):
                    for _ in range(min(n, len(fillers))):
                        fillers.pop(0)()

                # ---------- immediate per-chunk normalization: reciprocal of
                # ---------- the denominator row, K=1 ones-matmul broadcast
                # ---------- (ps_rep shares the psy "y" ring), multiply
                def emit_norm(h, c, y_c):
                    m = h // 2
                    pb = 64 * (h % 2)
                    cs = c * 512
                    rrow = rpool.tile([1, 512], bf16, tag="rrow",
                                      name=f"rr{h}{c}")
                    with nc.allow_low_precision(
                            reason="1/denom in bf16; ~0.4% y scale error is "
                                   "well inside the 2e-2 gate"):
                        nc.vector.reciprocal(rrow[:], y_c[64:65, :])
                    ps_rep = psy.tile([64, 512], f32, tag="y",
                                      name=f"rep{h}{c}")
                    nc.tensor.matmul(ps_rep[:], lhsT=ones64[:],
                                     rhs=rrow[:], start=True, stop=True)
                    nc.vector.tensor_tensor(
                        yN[pb:pb + 64, m, cs:cs + 512],
                        y_c[0:64, :], ps_rep[:], Alu.mult)

                # ---------- attention chunk-PAIR (q cols 1024p..1024p+1024):
                # ---------- each k-block's exp covers both 512-col q-chunks
                # ---------- in ONE ScalarE instruction (S matmuls fill the
                # ---------- two bank-halves of a [128,1024] psum tile); one
                # ---------- filler pumped per PV
                def emit_attn_pair(h, p):
                    m = h // 2
                    s = h % 2
                    pb = 64 * s
                    cl, cr = 2 * p, 2 * p + 1          # chunk indices
                    qs = 1024 * p
                    jmax_left = 8 * p + 3
                    nj = 8 * p + 8
                    ys_l = psy.tile([65, 512], f32, tag="y",
                                    name=f"ysl{h}{p}")
                    ys_r = psy.tile([65, 512], f32, tag="y",
                                    name=f"ysr{h}{p}")
                    y_cl = ypool.tile([65, 512], f32, tag="ysb",
                                      name=f"ycl{h}{p}")
                    y_cr = ypool.tile([65, 512], f32, tag="ysb",
                                      name=f"ycr{h}{p}")
                    pv_q = []          # deferred PV ops: (j, pt, side)
                    for j in range(nj):
                        left = j <= jmax_left
                        ps_st = psst.tile([128, 1024], f32, tag="st",
                                          name=f"st{h}{p}{j}")
                        if left:
                            # two matmuls fill the two bank-halves (a single
                            # 1024-wide psum write faults the PE: matmul
                            # output cannot cross a PSUM bank)
                            nc.tensor.matmul(
                                ps_st[:, 0:512],
                                lhsT=qkT[pb:pb + 64, 4 + m,
                                         j * 128:j * 128 + 128],
                                rhs=qkT[pb:pb + 64, m, qs:qs + 512],
                                start=True, stop=True)
                            nc.tensor.matmul(
                                ps_st[:, 512:1024],
                                lhsT=qkT[pb:pb + 64, 4 + m,
                                         j * 128:j * 128 + 128],
                                rhs=qkT[pb:pb + 64, m, qs + 512:qs + 1024],
                                start=True, stop=True)
                            jrel = j - 8 * p
                            if jrel <= 0:
                                pt = ptpool.tile([128, 1024], bf16, tag="pt",
                                                 name=f"pt{h}{p}{j}")
                                nc.scalar.activation(pt[:], ps_st[:], Act.Exp)
                            else:
                                pt = bands2[s][jrel - 1]
                                z = 128 * jrel
                                nc.scalar.activation(
                                    pt[:, z:1024], ps_st[:, z:1024], Act.Exp)
                            if jrel >= 0:
                                z = 128 * jrel
                                nc.gpsimd.tensor_tensor(
                                    pt[:, z:z + 128], pt[:, z:z + 128],
                                    mask_sb[:], Alu.mult)
                        else:
                            nc.tensor.matmul(
                                ps_st[:, 512:1024],
                                lhsT=qkT[pb:pb + 64, 4 + m,
                                         j * 128:j * 128 + 128],
                                rhs=qkT[pb:pb + 64, m, qs + 512:qs + 1024],
                                start=True, stop=True)
                            jrel = j - (8 * p + 4)     # right-chunk diagonal
                            if jrel <= 0:
                                pt = ptpool.tile([128, 1024], bf16, tag="pt",
                                                 name=f"pt{h}{p}{j}")
                                nc.scalar.activation(
                                    pt[:, 512:1024], ps_st[:, 512:1024],
                                    Act.Exp)
                            else:
                                pt = bands[s][jrel - 1]
                                z = 128 * jrel
                                nc.scalar.activation(
                                    pt[:, z:512], ps_st[:, 512 + z:1024],
                                    Act.Exp)
                            if jrel >= 0:
                                z = 128 * jrel
                                if pt.shape[1] == 1024:
                                    nc.gpsimd.tensor_tensor(
                                        pt[:, 512 + z:512 + z + 128],
                                        pt[:, 512 + z:512 + z + 128],
                                        mask_sb[:], Alu.mult)
                                else:
                                    nc.gpsimd.tensor_tensor(
                                        pt[:, z:z + 128], pt[:, z:z + 128],
                                        mask_sb[:], Alu.mult)

                        # drain the deferred-PV queue one block behind the
                        # exp frontier so PE never waits on a fresh exp
                        while pv_q and pv_q[0][0] < j:
                            pj, ppt, side = pv_q.pop(0)
                            pump(1)
                            if side == 'l':
                                nc.tensor.matmul(
                                    ys_l[:], lhsT=V[:, pj, h, :],
                                    rhs=ppt[:, 0:512],
                                    start=(pj == 0), stop=(pj == jmax_left))
                            else:
                                rw = (ppt[:, 512:1024]
                                      if ppt.shape[1] == 1024 else ppt[:])
                                nc.tensor.matmul(
                                    ys_r[:], lhsT=V[:, pj, h, :], rhs=rw,
                                    start=(pj == 0), stop=(pj == nj - 1))
                                if pj == jmax_left:
                                    # left accumulator complete: evacuate and
                                    # normalize mid-pair
                                    nc.vector.tensor_copy(y_cl[:], ys_l[:])
                                    emit_norm(h, cl, y_cl)
                        if left:
                            pv_q.append((j, pt, 'l'))
                        pv_q.append((j, pt, 'r'))
                    for pj, ppt, side in pv_q:
                        pump(1)
                        if side == 'l':
                            nc.tensor.matmul(
                                ys_l[:], lhsT=V[:, pj, h, :],
                                rhs=ppt[:, 0:512],
                                start=(pj == 0), stop=(pj == jmax_left))
                        else:
                            rw = (ppt[:, 512:1024]
                                  if ppt.shape[1] == 1024 else ppt[:])
                            nc.tensor.matmul(
                                ys_r[:], lhsT=V[:, pj, h, :], rhs=rw,
                                start=(pj == 0), stop=(pj == nj - 1))
                            if pj == jmax_left:
                                nc.vector.tensor_copy(y_cl[:], ys_l[:])
                                emit_norm(h, cl, y_cl)
                    nc.vector.tensor_copy(y_cr[:], ys_r[:])
                    emit_norm(h, cr, y_cr)

                # ---------- schedule ----------
                # preamble: qk quarters 0-1 + V token-blocks 0-7 (everything
                # the first head-pair's chunk-pair 0 touches)
                emit_qk_tile(0, 0)
                emit_qk_tile(4, 0)
                emit_qk_tile(0, 1)
                emit_qk_tile(4, 1)
                for tb in range(8):
                    emit_v_tile(tb)

                for g in range(4):          # head pairs (2g, 2g+1)
                    if g > 0:
                        # fillers for this pair were queued during pair g-1;
                        # drain any leftovers before their consumers
                        while fillers:
                            pump(1)
                    if g < 3:
                        for mm in (g + 1, 4 + g + 1):
                            for tq in range(4):
                                queue_qk_tile(mm, tq)
                    emit_attn_pair(2 * g, 0)
                    emit_attn_pair(2 * g + 1, 0)
                    if g == 0:
                        # pair-0's own later quarters + V blocks (burst:
                        # consumers are too close for filler pacing)
                        emit_qk_tile(0, 2)
                        emit_qk_tile(4, 2)
                        emit_qk_tile(0, 3)
                        emit_qk_tile(4, 3)
                        for tb in range(8, 16):
                            emit_v_tile(tb)
                    if g == 3:
                        # chunks 0-1 of all heads normalized after h7 pair 0
                        for t in range(8):
                            queue_proj_tile(0, t)
                            queue_proj_tile(1, t)
                    emit_attn_pair(2 * g, 1)
                    emit_attn_pair(2 * g + 1, 1)

                # chunks 2-3 of all heads are now normalized
                for t in range(8, 16):
                    queue_proj_tile(0, t)
                    queue_proj_tile(1, t)
                # drain remaining projection fillers
                while fillers:
                    pump(1)

    nsplit = _split_multiwaits(nc)
    return nc, nsplit


def _prep_inputs(x, w_attn, b_attn, w_proj):
    """Per-core input maps. Core c: batch c//2, head-group c%2."""
    import ml_dtypes
    bf = ml_dtypes.bfloat16
    x = np.ascontiguousarray(x, dtype=np.float32)
    w_attn = np.asarray(w_attn, dtype=np.float32)
    b_attn = np.asarray(b_attn, dtype=np.float32)
    w_proj = np.asarray(w_proj, dtype=np.float32)
    scale = np.float32(1.0 / np.sqrt(HD))

    mask = (np.arange(128)[:, None] <= np.arange(128)[None, :]).astype(bf)

    in_maps = []
    for core in range(NCORES):
        b = core // 2
        g = core % 2
        gc = CL * g
        wq = w_attn[gc:gc + CL, :] * scale          # [512, 1024]
        wk = w_attn[C + gc:C + gc + CL, :]
        wv = w_attn[2 * C + gc:2 * C + gc + CL, :]
        bq = b_attn[gc:gc + CL] * scale
        in_maps.append({
            "xT": np.ascontiguousarray(x[b].T).astype(bf),
            "wqk": np.ascontiguousarray(
                np.concatenate([wq.T, wk.T], axis=1)).astype(bf),
            "wv": np.ascontiguousarray(wv.T).astype(bf),
            "bq": np.ascontiguousarray(bq.reshape(4, 128).T),
            "wp": np.ascontiguousarray(
                w_proj[:, gc:gc + CL].T.astype(bf)),
            "mask": mask,
        })
    return in_maps


def _run(in_maps, reps=1):
    from concourse.bass_utils import run_bass_kernel_spmd
    key = reps
    if key not in _cache:
        _cache[key] = _build(reps)
    nc, _ = _cache[key]
    return run_bass_kernel_spmd(nc, in_maps, list(range(NCORES)))


def kernel(x, w_attn, b_attn, w_proj, b_proj):
    x = np.asarray(x, dtype=np.float32)
    w_attn = np.asarray(w_attn, dtype=np.float32)
    b_attn = np.asarray(b_attn, dtype=np.float32)
    w_proj = np.asarray(w_proj, dtype=np.float32)
    b_proj = np.asarray(b_proj, dtype=np.float32)

    in_maps = _prep_inputs(x, w_attn, b_attn, w_proj)
    res = _run(in_maps).results

    # host-side unshard: sum the two head-group partials per batch and add
    # the bias terms (b_proj + w_proj @ b_v; softmax rows sum to 1).
    bv = b_attn[2 * C:]
    const = (w_proj @ bv + b_proj).astype(np.float32)
    out = np.empty((B, T, C), dtype=np.float32)
    for b in range(B):
        out[b] = (res[2 * b]["out"].astype(np.float32)
                  + res[2 * b + 1]["out"].astype(np.float32) + const)
    return out
